# revision 21
# baseline (speedup 1.0000x reference)
"""Bregman-ADMM iteration kernel for Trainium2, 8-core data-parallel.

Problem: 20 iterations of
    R = softmax((log R0' + log X' - Z) / 2, axis=-1)      (row softmax, N=1024)
    x = R + Z ; s = relu(x - 0.05)                        (x > 0 always, provable)
    col_norm[n] = sqrt(sum_{B,M} s^2)                     (global -> AllGather)
    scale = relu(1 - 0.05 / (col_norm + 1e-10))
    X = scale * s ; Z = x - X
Output: final R.  (X' = max(X, 1e-40), matching where(X==0, 1e-40, X) for X>=0.)

Sharding: batch dim B=16 split across 8 cores (2 batches = 2048 rows per core).
Per-core state in SBUF: lx = log X' (8 MB), zx = Z/x (8 MB); L0 = log R0'
streamed from HBM each iteration. Cross-core: one AllGather of the [1024]
partial column sum-of-squares per iteration.
"""

import os
import numpy as np

import concourse.bass as bass
import concourse.bacc as bacc
import concourse.mybir as mybir
import concourse.tile as tile
from concourse import bass_utils

F32 = mybir.dt.float32
AF = mybir.ActivationFunctionType
ALU = mybir.AluOpType
AX = mybir.AxisListType

N_CORES = 8
N = 1024            # column dim (softmax axis)
ROWS = 2048         # rows per core = (16/8 batches) * 1024
P = 128             # partitions
NT = ROWS // P      # 16 row-tiles per core
N_IT = int(os.environ.get("BREG_NIT", "20"))
HALF = 512          # fp32 matmul moving-operand max

MIN_VALUE = 1e-40
# Clamp constant for log inputs. ACT's Ln spline is only accurate on
# ~[1e-16, 1e16]; live X values are always 0 or >= ~2e-16, so the clamp only
# hits exact zeros. Its device Ln value (-45.598, vs true -46.356) is
# consistent across elements, and a row-uniform shift cancels in the softmax;
# in mixed rows clamped elements carry e^-5 relative weight (reference: ~0).
CLAMP = 7.378697629483821e-21
EPSILON = 1e-10
THR = 0.05          # ALPHA*LAMBD/RHO
COEF = 0.05         # (1-ALPHA)*LAMBD/RHO


def fill_diag(nc, ap, val):
    nc.gpsimd.memset(ap, 0.0)
    nc.gpsimd.affine_select(
        out=ap, in_=ap, compare_op=ALU.not_equal, fill=val,
        base=0, pattern=[[-1, P]], channel_multiplier=1,
    )


DEBUG_STATE = bool(int(os.environ.get("BREG_DEBUG", "0")))


def build(n_it=N_IT, nt=NT):
    nc = bacc.Bacc("TRN2", target_bir_lowering=False, debug=False,
                   num_devices=N_CORES)
    rows = nt * P
    r0 = nc.dram_tensor("r0", [rows, N], F32, kind="ExternalInput").ap()
    r_out = nc.dram_tensor("r_out", [rows, N], F32, kind="ExternalOutput").ap()
    if DEBUG_STATE:
        zx_out = nc.dram_tensor("zx_out", [P, nt * N], F32,
                                kind="ExternalOutput").ap()
        lx_out = nc.dram_tensor("lx_out", [P, nt * N], F32,
                                kind="ExternalOutput").ap()
        t_out = nc.dram_tensor("t_out", [P, N], F32,
                               kind="ExternalOutput").ap()
        e_out = nc.dram_tensor("e_out", [P, N], F32,
                               kind="ExternalOutput").ap()
        l0t_out = nc.dram_tensor("l0t_out", [P, N], F32,
                                 kind="ExternalOutput").ap()

    with tile.TileContext(nc) as tc:
        with (
            tc.tile_pool(name="state", bufs=1) as state_pool,
            tc.tile_pool(name="consts", bufs=1) as consts,
            tc.tile_pool(name="io", bufs=2) as io,
            tc.tile_pool(name="work", bufs=3) as work,
            tc.tile_pool(name="small", bufs=4) as small,
            tc.tile_pool(name="vecp", bufs=3) as vecp,
            tc.tile_pool(name="tps", bufs=2, space="PSUM") as tps,
            tc.tile_pool(name="stat", bufs=2, space="PSUM") as stat,
            tc.tile_pool(name="dram", bufs=1, space="DRAM") as dram,
            tc.tile_pool(name="ccp", bufs=2, space="DRAM") as ccp,
        ):
            # persistent state
            lx = state_pool.tile([P, nt * N], F32, name="lx")        # log X'
            zx = state_pool.tile([P, nt * N], F32, name="zx")        # Z, then x
            l0_dram = dram.tile([rows, N], F32, name="l0_dram")

            # constants
            halfI = consts.tile([P, P], F32, name="halfI")
            neghalfI = consts.tile([P, P], F32, name="neghalfI")
            fill_diag(nc, halfI[:], 0.5)
            fill_diag(nc, neghalfI[:], -0.5)
            ones_p1 = consts.tile([P, 1], F32, name="ones_p1")
            nc.vector.memset(ones_p1[:], 1.0)
            ones_1p = consts.tile([1, P], F32, name="ones_1p")
            nc.vector.memset(ones_1p[:], 1.0)
            ones_81 = consts.tile([N_CORES, 1], F32, name="ones_81")
            nc.vector.memset(ones_81[:], 1.0)

            # ---- prologue: lx = L0 = log(max(r0, MIN)); l0_dram = L0; Z = 0
            nc.vector.memset(zx[:], 0.0)
            for i in range(nt):
                sl = slice(i * N, (i + 1) * N)
                rsl = slice(i * P, (i + 1) * P)
                rt = io.tile([P, N], F32, name="rt", tag="io")
                nc.sync.dma_start(rt[:], r0[rsl, :])
                nc.vector.tensor_scalar(rt[:], rt[:], CLAMP, None, ALU.max)
                nc.scalar.activation(lx[:, sl], rt[:], AF.Ln)
                nc.sync.dma_start(l0_dram[rsl, :], lx[:, sl])

            # ---- iterations
            for k in range(n_it):
                last = k == n_it - 1
                colsum = None if last else stat.tile([1, N], F32, tag="stat",
                                                     name="colsum")
                for i in range(nt):
                    sl = slice(i * N, (i + 1) * N)
                    rsl = slice(i * P, (i + 1) * P)
                    l0_t = io.tile([P, N], F32, name="l0_t", tag="io")
                    nc.sync.dma_start(l0_t[:], l0_dram[rsl, :])
                    # t = 0.5*(L0 + lx - Z)   (PE identity-accumulate, PSUM)
                    t_ps = tps.tile([P, N], F32, name="t_ps")
                    for h in range(N // HALF):
                        hs = slice(h * HALF, (h + 1) * HALF)
                        hsl = slice(i * N + h * HALF, i * N + (h + 1) * HALF)
                        nc.tensor.matmul(t_ps[:, hs], halfI[:], l0_t[:, hs],
                                         start=True, stop=False)
                        nc.tensor.matmul(t_ps[:, hs], halfI[:], lx[:, hsl],
                                         start=False, stop=False)
                        nc.tensor.matmul(t_ps[:, hs], neghalfI[:], zx[:, hsl],
                                         start=False, stop=True)
                    # row softmax (no max-subtraction: t in [-56, 2], exp is
                    # accurate and in fp32 range over the whole interval)
                    e_t = work.tile([P, N], F32, name="e_t")
                    sig = small.tile([P, 1], F32, name="sig")
                    nc.scalar.activation(e_t[:], t_ps[:], AF.Exp,
                                         accum_out=sig[:])
                    rsig = small.tile([P, 1], F32, name="rsig")
                    nc.vector.reciprocal(rsig[:], sig[:])
                    if DEBUG_STATE and last and i == 0:
                        tdbg = work.tile([P, N], F32, name="tdbg", bufs=1)
                        nc.vector.tensor_copy(tdbg[:], t_ps[:])
                        nc.sync.dma_start(t_out[:], tdbg[:])
                        nc.sync.dma_start(e_out[:], e_t[:])
                        nc.sync.dma_start(l0t_out[:], l0_t[:])
                    # R = e / sum  (in place over e_t)
                    nc.vector.tensor_scalar(e_t[:], e_t[:], rsig[:], None,
                                            ALU.mult)
                    if last:
                        nc.sync.dma_start(r_out[rsl, :], e_t[:])
                        continue
                    # x = R + Z  (into zx slot)
                    nc.vector.tensor_tensor(zx[:, sl], e_t[:], zx[:, sl],
                                            ALU.add)
                    # s = relu(x - THR); q = s^2 (in place); colsum += ones^T q
                    s_t = work.tile([P, N], F32, name="s_t")
                    nc.gpsimd.tensor_scalar(s_t[:], zx[:, sl], THR, 0.0,
                                            ALU.subtract, ALU.max)
                    nc.scalar.activation(s_t[:], s_t[:], AF.Square)
                    for h in range(N // HALF):
                        hs = slice(h * HALF, (h + 1) * HALF)
                        nc.tensor.matmul(colsum[:, hs], ones_p1[:], s_t[:, hs],
                                         start=(i == 0), stop=(i == nt - 1),
                                         skip_group_check=True)
                if last:
                    break

                # ---- cross-core: AllGather partial column sums
                cn_sb = vecp.tile([1, N], F32, name="cn_sb", tag="vec")
                nc.scalar.copy(cn_sb[:], colsum[:])
                cc_in = ccp.tile([1, N], F32, name="cc_in")
                cc_out = ccp.tile([N_CORES, N], F32, name="cc_out",
                                  addr_space="Shared")
                nc.sync.dma_start(cc_in[:], cn_sb[:])
                nc.gpsimd.collective_compute(
                    "AllGather", ALU.bypass,
                    replica_groups=[list(range(N_CORES))],
                    ins=[cc_in[:]], outs=[cc_out[:]],
                )
                g_sb = vecp.tile([N_CORES, N], F32, name="g_sb", tag="vec")
                nc.sync.dma_start(g_sb[:], cc_out[:])
                norm2 = stat.tile([1, N], F32, tag="stat", name="norm2")
                for h in range(N // HALF):
                    hs = slice(h * HALF, (h + 1) * HALF)
                    nc.tensor.matmul(norm2[:, hs], ones_81[:], g_sb[:, hs],
                                     start=True, stop=True)
                # scale = relu(1 - COEF / (sqrt(norm2) + EPSILON))
                nrm = vecp.tile([1, N], F32, name="nrm", tag="vec")
                nc.scalar.activation(nrm[:], norm2[:], AF.Sqrt)
                nc.vector.tensor_scalar(nrm[:], nrm[:], EPSILON, None, ALU.add)
                rnrm = vecp.tile([1, N], F32, name="rnrm", tag="vec")
                nc.vector.reciprocal(rnrm[:], nrm[:])
                scale_sb = vecp.tile([1, N], F32, name="scale_sb", tag="vec")
                nc.scalar.activation(scale_sb[:], rnrm[:], AF.Relu,
                                     bias=1.0, scale=-COEF)
                # broadcast scale to all 128 partitions
                scb_ps = stat.tile([P, N], F32, tag="stat", name="scb_ps")
                for h in range(N // HALF):
                    hs = slice(h * HALF, (h + 1) * HALF)
                    nc.tensor.matmul(scb_ps[:, hs], ones_1p[:],
                                     scale_sb[:, hs], start=True, stop=True)
                scb_sb = work.tile([P, N], F32, name="scb_sb", tag="scb",
                                   bufs=2)
                nc.scalar.copy(scb_sb[:], scb_ps[:])

                # ---- stage C: X = scale*s ; Z' = x - X ; lx' = log(max(X,MIN))
                for i in range(nt):
                    sl = slice(i * N, (i + 1) * N)
                    s_t = work.tile([P, N], F32, name="s_t")
                    nc.gpsimd.tensor_scalar(s_t[:], zx[:, sl], THR, 0.0,
                                            ALU.subtract, ALU.max)
                    x_t = work.tile([P, N], F32, name="x_t", bufs=2)
                    nc.vector.tensor_tensor(x_t[:], s_t[:], scb_sb[:], ALU.mult)
                    nc.vector.tensor_tensor(zx[:, sl], zx[:, sl], x_t[:],
                                            ALU.subtract)
                    nc.gpsimd.tensor_scalar(x_t[:], x_t[:], CLAMP, None,
                                            ALU.max)
                    nc.scalar.activation(lx[:, sl], x_t[:], AF.Ln)

            if DEBUG_STATE:
                nc.sync.dma_start(zx_out[:], zx[:])
                nc.sync.dma_start(lx_out[:], lx[:])

    nc.compile()
    return nc


_CACHE = {}


def _get_nc():
    key = (N_IT, NT)
    if key not in _CACHE:
        _CACHE[key] = build(N_IT, NT)
    return _CACHE[key]


def kernel(R_0: np.ndarray) -> np.ndarray:
    R_0 = np.ascontiguousarray(np.asarray(R_0, dtype=np.float32))
    B, M, Ncols = R_0.shape
    per = B // N_CORES
    nc = _get_nc()
    in_maps = [
        {"r0": R_0[c * per:(c + 1) * per].reshape(per * M, Ncols)}
        for c in range(N_CORES)
    ]
    res = bass_utils.run_bass_kernel_spmd(
        nc, in_maps, core_ids=list(range(N_CORES)),
        trace=bool(int(os.environ.get("BREG_TRACE", "0"))),
    )
    out = np.concatenate(
        [res.results[c]["r_out"].reshape(per, M, Ncols)
         for c in range(N_CORES)], axis=0)
    kernel.last_exec_time_ns = res.exec_time_ns
    kernel.last_results = res
    return out


if __name__ == "__main__":
    rng = np.random.default_rng(0)
    R_0 = rng.random((16, 1024, 1024), dtype=np.float32)
    out = kernel(R_0)
    print("out", out.shape, out.dtype, out.min(), out.max())
